# revision 29
# baseline (speedup 1.0000x reference)
"""Attention-pooling kernel (AttLayer) for Trainium2, 8 NeuronCores.

Math (per batch b):
    uit  = tanh(x @ W + b)          # [T, A]
    ait  = exp(uit @ u)             # [T]
    out  = (sum_t ait[t] * x[t,:]) / (sum_t ait[t] + EPS)   # [D]

Single pass over x; numerator accumulated on DVE (custom
TENSOR_TENSOR_REDUCE, 1x mode -> the kernel is DVE-bound), so the
design minimizes DVE op count (one [128, 4096] TTR per (batch, d-chunk))
and keeps every other engine far below the DVE budget:

  - stage1 (uit^T) is 2-way "stacked": even 512-token blocks land on
    psum partitions 0-63, odd blocks on 64-127 (weights zero-padded to
    64 columns so all rows are written -> no NaN garbage). This halves
    ACT tanh time (free-dim is what costs on ACT).
  - stage2 (logit broadcast) is row-packed: two concurrent K=64
    matmuls (tile_position (0,0)/(64,0) inferred from base partitions)
    produce the [128, t] broadcast logits for two 512-blocks at once.
  - exp materializes e (bf16) into a [128, 4096] per-batch tile with
    accum_out -> per-e-tile denominator partials.

Device data layout (per core, pure data-parallel over batch; host casts
to bf16 and pre-transposes so x streams with d on partitions):
    xt   [2, 128, BL*T] bf16  -- x^T: (d_chunk, d_in_chunk, b*t)
    w    [2, 128, 64]   bf16  -- W d-chunks, zero-padded cols 50..63
    bb   [128, 1]       f32   -- bias stacked (rows 0-49 & 64-113 = b)
    urep [128, 128]     bf16  -- u bcast: rows 0-49 & 64-113 = u, else 0
Outputs:
    num  [128, 2*BL]    f32   -- numerator, col = b*2 + c
    den  [1, 4*BL]      f32   -- per-e-tile exp-sum partials, col = b*4+k
Final division num/(den+EPS) happens on host during unsharding.
"""

import sys
import types

sys.path.insert(0, "/opt/trn_rl_repo")

# bass_utils' trace path imports antenv.axon_hooks, which not every image
# ships; register a no-op fallback so trace=True degrades instead of crashing.
try:
    import antenv.axon_hooks  # noqa: F401
except ImportError:
    try:
        import antenv

        _hooks = types.ModuleType("antenv.axon_hooks")
        _hooks._HOOK = None

        def _set_hook(hook):
            _hooks._HOOK = hook

        def _get_hook():
            return _hooks._HOOK

        _hooks.set_axon_ntff_profile_hook = _set_hook
        _hooks.get_axon_ntff_profile_hook = _get_hook
        sys.modules["antenv.axon_hooks"] = _hooks
        antenv.axon_hooks = _hooks
    except ImportError:
        pass

import numpy as np
import ml_dtypes

import concourse.bacc as bacc
import concourse.tile as tile
from concourse import mybir
from concourse import bass_utils
from concourse.dve_ops import TENSOR_TENSOR_REDUCE

B, T, D, A = 64, 4096, 256, 50
NCORES = 8
BL = B // NCORES  # batches per core
EPS = 1e-7
P = 128
NCH = D // P  # 2 d-chunks
NE = 4        # e-tiles (of 1024 tokens) per batch
# Batches whose numerator runs on the Tensor engine (from a second,
# natural-layout x stream) instead of the DVE: the DVE's 1x-mode
# TENSOR_TENSOR_REDUCE is the kernel bottleneck, so offloading 2 of 8
# batches re-balances DVE (~54us) against DMA (~53us) / ACT (~56us).
# Disabled (PEB = ()): measured 101.5us vs 98.6us for the pure-DVE
# pipeline, and first-run-after-compile reliability was worse. The
# machinery is kept behind the PEB flag for future tuning.
PEB = ()
NPEB = len(PEB)
NJ = T // P   # 32 token groups of 128 per batch


def build_attpool(nc, aps):
    xt, w, bb, urep = aps["xt"], aps["w"], aps["bb"], aps["urep"]
    num, den = aps["num"], aps["den"]
    f32 = mybir.dt.float32
    bf16 = mybir.dt.bfloat16
    Tanh = mybir.ActivationFunctionType.Tanh
    Exp = mybir.ActivationFunctionType.Exp

    with tile.TileContext(nc) as tc:
        with (
            tc.tile_pool(name="singles", bufs=1) as singles,
            tc.tile_pool(name="x0", bufs=4) as x0_pool,
            tc.tile_pool(name="x1", bufs=4) as x1_pool,
            tc.tile_pool(name="uitT", bufs=2) as uitT_pool,
            tc.tile_pool(name="e", bufs=3) as e_pool,
            tc.tile_pool(name="scratch", bufs=3) as scratch_pool,
            tc.tile_pool(name="ps_uit", bufs=2, space="PSUM") as ps_uit_pool,
            tc.tile_pool(name="ps_e", bufs=2, space="PSUM") as ps_e_pool,
        ):
            # constants (tiles allocated first so SBUF layout is stable;
            # their DMA issues go out after the first x pieces -- see below)
            w_sb = [
                singles.tile([P, 64], bf16, tag=f"w{c}", name=f"w_sb{c}")
                for c in range(NCH)
            ]
            bb_sb = singles.tile([P, 1], f32)
            urep_sb = singles.tile([P, P], bf16)
            num_sb = singles.tile([P, NCH * BL], f32)
            den_sb = singles.tile([P, NE * BL], f32)

            for c in range(NCH):
                nc.sync.dma_start(out=w_sb[c][:, :], in_=w[c, :, :])
            nc.sync.dma_start(out=bb_sb[:, :], in_=bb[:, :])
            nc.sync.dma_start(out=urep_sb[:, :], in_=urep[:, :])

            x_pools = (x0_pool, x1_pool)
            def batch(b):
                # DMA both chunk tiles for this batch (1MB each). The first
                # batch splits into 1024-col pieces, interleaved across the
                # chunks, so stage1 of the first token group can start after
                # ~0.5MB instead of 2MB (shorter pipeline ramp).
                xt_t = []
                for c in range(NCH):
                    t_ = x_pools[c].tile([P, T], bf16, tag=f"xt{c}", name=f"xt{c}")
                    xt_t.append(t_)
                if b == 0:
                    # 1024-col pieces so stage1 starts after ~0.5MB
                    for k in range(NE):
                        for c in range(NCH):
                            nc.sync.dma_start(
                                out=xt_t[c][:, k * 1024 : (k + 1) * 1024],
                                in_=xt[c, :, k * 1024 : (k + 1) * 1024],
                            )
                else:
                    for c in range(NCH):
                        nc.sync.dma_start(
                            out=xt_t[c][:, :], in_=xt[c, :, b * T : (b + 1) * T]
                        )
                e_sb = e_pool.tile([P, T], bf16, tag="e", name="e_sb")
                uitT = [None, None]
                for k in range(NE):  # e-tile = 1024 tokens
                    g, r = divmod(k, 2)
                    if r == 0:
                        # stage1 for token group g (2048 tokens, stacked)
                        U = ps_uit_pool.tile([P, 1024], f32, tag="U", name="U")
                        for rr in range(2):
                            t0 = g * 2048 + rr * 1024
                            for c in range(NCH):
                                # even 512-block -> partitions 0:64
                                nc.tensor.matmul(
                                    U[0:64, rr * 512 : (rr + 1) * 512],
                                    lhsT=w_sb[c][:, :],
                                    rhs=xt_t[c][:, t0 : t0 + 512],
                                    start=(c == 0),
                                    stop=(c == NCH - 1),
                                )
                                # odd 512-block -> partitions 64:128
                                nc.tensor.matmul(
                                    U[64:128, rr * 512 : (rr + 1) * 512],
                                    lhsT=w_sb[c][:, :],
                                    rhs=xt_t[c][:, t0 + 512 : t0 + 1024],
                                    start=(c == 0),
                                    stop=(c == NCH - 1),
                                )
                        uitT[g] = uitT_pool.tile([P, 1024], bf16, tag="uitT", name="uitT")
                        nc.scalar.activation(
                            uitT[g][:, :], U[:, :], Tanh, bias=bb_sb[:, :]
                        )
                    # stage2: two row-packed matmuls -> [128, 1024] logits
                    E = ps_e_pool.tile([P, 1024], f32, tag="E", name="E")
                    nc.tensor.matmul(
                        E[:, 0:512],
                        lhsT=urep_sb[0:64, :],
                        rhs=uitT[g][0:64, r * 512 : (r + 1) * 512],
                        start=True,
                        stop=True,
                    )
                    nc.tensor.matmul(
                        E[:, 512:1024],
                        lhsT=urep_sb[64:128, :],
                        rhs=uitT[g][64:128, r * 512 : (r + 1) * 512],
                        start=True,
                        stop=True,
                    )
                    nc.scalar.activation(
                        e_sb[:, k * 1024 : (k + 1) * 1024], E[:, :], Exp,
                        accum_out=den_sb[:, b * NE + k : b * NE + k + 1],
                    )
                # numerator TTRs on DVE (the bottleneck engine).
                # batch 0 runs per-e-tile with seed chaining so the DVE can
                # start before the whole batch's e is ready (shorter ramp);
                # later batches use one big TTR per chunk (lower overhead).
                if b == 0:
                    for c in range(NCH):
                        col = b * NCH + c
                        for k in range(NE):
                            scr = scratch_pool.tile(
                                [P, 1024], bf16, tag=f"scr{c}", name="scr_k"
                            )
                            nc.vector._custom_dve(
                                TENSOR_TENSOR_REDUCE,
                                out=scr[:, :],
                                in0=xt_t[c][:, k * 1024 : (k + 1) * 1024],
                                in1=e_sb[:, k * 1024 : (k + 1) * 1024],
                                s0=0.0 if k == 0 else num_sb[:, col : col + 1],
                                s1=1.0,
                                accum_out=num_sb[:, col : col + 1],
                            )
                else:
                    for c in range(NCH):
                        col = b * NCH + c
                        scr = scratch_pool.tile(
                            [P, T], bf16, tag=f"scr{c}", name="scr_b"
                        )
                        nc.vector._custom_dve(
                            TENSOR_TENSOR_REDUCE,
                            out=scr[:, :],
                            in0=xt_t[c][:, :],
                            in1=e_sb[:, :],
                            s0=0.0,
                            s1=1.0,
                            accum_out=num_sb[:, col : col + 1],
                        )

            for b in range(BL):
                batch(b)
                # drain this batch's numerator right away: keeps the tail
                # DMA tiny and avoids a bulk sync point near the end
                nc.sync.dma_start(
                    out=num[:, b * NCH : (b + 1) * NCH],
                    in_=num_sb[:, b * NCH : (b + 1) * NCH],
                )
            nc.sync.dma_start(out=den[:, :], in_=den_sb[0:1, :])
    return nc


def _declare(nc):
    f32 = mybir.dt.float32
    bf16 = mybir.dt.bfloat16
    aps = {
        "xt": nc.dram_tensor("xt", (NCH, P, BL * T), bf16, kind="ExternalInput").ap(),
        "w": nc.dram_tensor("w", (NCH, P, 64), bf16, kind="ExternalInput").ap(),
        "bb": nc.dram_tensor("bb", (P, 1), f32, kind="ExternalInput").ap(),
        "urep": nc.dram_tensor("urep", (P, P), bf16, kind="ExternalInput").ap(),
        "num": nc.dram_tensor("num", (P, NCH * BL), f32, kind="ExternalOutput").ap(),
        "den": nc.dram_tensor("den", (1, NE * BL), f32, kind="ExternalOutput").ap(),
    }
    return aps


_CACHE = {}


def _get_nc():
    key = "nc"
    if key not in _CACHE:
        nc = bacc.Bacc(
            "TRN2", target_bir_lowering=False, debug=False,
            enable_asserts=False, num_devices=NCORES,
        )
        aps = _declare(nc)
        build_attpool(nc, aps)
        nc.compile()
        _CACHE[key] = nc
    return _CACHE[key]


def _host_prep(x, W, b, u):
    """Build per-core input maps from full inputs."""
    x = np.asarray(x, dtype=np.float32)
    W = np.asarray(W, dtype=np.float32)
    b = np.asarray(b, dtype=np.float32)
    u = np.asarray(u, dtype=np.float32)
    wc = np.zeros((NCH, P, 64), dtype=np.float32)
    wc[:, :, :A] = W.reshape(NCH, P, A)
    wc = wc.astype(ml_dtypes.bfloat16)
    bb = np.zeros((P, 1), dtype=np.float32)
    bb[0:A, 0] = b
    bb[64 : 64 + A, 0] = b
    urep = np.zeros((P, P), dtype=np.float32)
    urep[0:A, :] = u.reshape(A, 1)
    urep[64 : 64 + A, :] = u.reshape(A, 1)
    urep = urep.astype(ml_dtypes.bfloat16)
    in_maps = []
    for core in range(NCORES):
        xc = x[core * BL : (core + 1) * BL]  # [BL, T, D]
        # -> [NCH, P, BL*T]: xt[c, dp, b*T+t] = x[b, t, c*128+dp]
        xt = np.ascontiguousarray(
            xc.reshape(BL, T, NCH, P).transpose(2, 3, 0, 1).reshape(NCH, P, BL * T)
        ).astype(ml_dtypes.bfloat16)
        in_maps.append({"xt": xt, "w": wc, "bb": bb, "urep": urep})
    return in_maps


def _unshard(results):
    out = np.empty((B, D), dtype=np.float32)
    for core in range(NCORES):
        num = results[core]["num"]          # [128, 2*BL]
        den = results[core]["den"]          # [1, NE*BL]
        den_b = den.reshape(BL, NE).sum(axis=1)  # [BL]
        for bl in range(BL):
            vec = np.concatenate(
                [num[:, bl * NCH + c] for c in range(NCH)]
            )  # [D]
            out[core * BL + bl] = vec / (den_b[bl] + EPS)
    return out


def kernel(x, W, b, u, _trace=False):
    nc = _get_nc()
    in_maps = _host_prep(x, W, b, u)
    res = bass_utils.run_bass_kernel_spmd(
        nc, in_maps, core_ids=list(range(NCORES)), trace=_trace,
    )
    out = _unshard(res.results)
    if _trace:
        kernel.last_result = res
    return out


# revision 30
# speedup vs baseline: 1.0147x; 1.0147x over previous
"""Attention-pooling kernel (AttLayer) for Trainium2, 8 NeuronCores.

Math (per batch b):
    uit  = tanh(x @ W + b)          # [T, A]
    ait  = exp(uit @ u)             # [T]
    out  = (sum_t ait[t] * x[t,:]) / (sum_t ait[t] + EPS)   # [D]

Single pass over x; numerator accumulated on DVE (custom
TENSOR_TENSOR_REDUCE, 1x mode -> the kernel is DVE-bound), so the
design minimizes DVE op count (one [128, 4096] TTR per (batch, d-chunk))
and keeps every other engine far below the DVE budget:

  - stage1 (uit^T) is 2-way "stacked": even 512-token blocks land on
    psum partitions 0-63, odd blocks on 64-127 (weights zero-padded to
    64 columns so all rows are written -> no NaN garbage). This halves
    ACT tanh time (free-dim is what costs on ACT).
  - stage2 (logit broadcast) is row-packed: two concurrent K=64
    matmuls (tile_position (0,0)/(64,0) inferred from base partitions)
    produce the [128, t] broadcast logits for two 512-blocks at once.
  - exp materializes e (bf16) into a [128, 4096] per-batch tile with
    accum_out -> per-e-tile denominator partials.

Device data layout (per core, pure data-parallel over batch; host casts
to bf16 and pre-transposes so x streams with d on partitions):
    xt   [2, 128, BL*T] bf16  -- x^T: (d_chunk, d_in_chunk, b*t)
    w    [2, 128, 64]   bf16  -- W d-chunks, zero-padded cols 50..63
    bb   [128, 1]       f32   -- bias stacked (rows 0-49 & 64-113 = b)
    urep [128, 128]     bf16  -- u bcast: rows 0-49 & 64-113 = u, else 0
Outputs:
    num  [128, 2*BL]    f32   -- numerator, col = b*2 + c
    den  [1, 4*BL]      f32   -- per-e-tile exp-sum partials, col = b*4+k
Final division num/(den+EPS) happens on host during unsharding.
"""

import sys
import types

sys.path.insert(0, "/opt/trn_rl_repo")

# bass_utils' trace path imports antenv.axon_hooks, which not every image
# ships; register a no-op fallback so trace=True degrades instead of crashing.
try:
    import antenv.axon_hooks  # noqa: F401
except ImportError:
    try:
        import antenv

        _hooks = types.ModuleType("antenv.axon_hooks")
        _hooks._HOOK = None

        def _set_hook(hook):
            _hooks._HOOK = hook

        def _get_hook():
            return _hooks._HOOK

        _hooks.set_axon_ntff_profile_hook = _set_hook
        _hooks.get_axon_ntff_profile_hook = _get_hook
        sys.modules["antenv.axon_hooks"] = _hooks
        antenv.axon_hooks = _hooks
    except ImportError:
        pass

import numpy as np
import ml_dtypes

import concourse.bacc as bacc
import concourse.tile as tile
from concourse import mybir
from concourse import bass_utils
from concourse.dve_ops import TENSOR_TENSOR_REDUCE

B, T, D, A = 64, 4096, 256, 50
NCORES = 8
BL = B // NCORES  # batches per core
EPS = 1e-7
P = 128
NCH = D // P  # 2 d-chunks
NE = 4        # e-tiles (of 1024 tokens) per batch
# Batches whose numerator runs on the Tensor engine (from a second,
# natural-layout x stream) instead of the DVE: the DVE's 1x-mode
# TENSOR_TENSOR_REDUCE is the kernel bottleneck, so offloading 2 of 8
# batches re-balances DVE (~54us) against DMA (~53us) / ACT (~56us).
# Disabled (PEB = ()): measured 101.5us vs 98.6us for the pure-DVE
# pipeline, and first-run-after-compile reliability was worse. The
# machinery is kept behind the PEB flag for future tuning.
PEB = ()
NPEB = len(PEB)
NJ = T // P   # 32 token groups of 128 per batch


def build_attpool(nc, aps):
    xt, w, bb, urep = aps["xt"], aps["w"], aps["bb"], aps["urep"]
    num, den = aps["num"], aps["den"]
    f32 = mybir.dt.float32
    bf16 = mybir.dt.bfloat16
    Tanh = mybir.ActivationFunctionType.Tanh
    Exp = mybir.ActivationFunctionType.Exp

    with tile.TileContext(nc) as tc:
        with (
            tc.tile_pool(name="singles", bufs=1) as singles,
            tc.tile_pool(name="x0", bufs=4) as x0_pool,
            tc.tile_pool(name="x1", bufs=4) as x1_pool,
            tc.tile_pool(name="uitT", bufs=2) as uitT_pool,
            tc.tile_pool(name="e", bufs=3) as e_pool,
            tc.tile_pool(name="scratch", bufs=3) as scratch_pool,
            tc.tile_pool(name="ps_uit", bufs=2, space="PSUM") as ps_uit_pool,
            tc.tile_pool(name="ps_e", bufs=2, space="PSUM") as ps_e_pool,
        ):
            # constants (tiles allocated first so SBUF layout is stable;
            # their DMA issues go out after the first x pieces -- see below)
            w_sb = [
                singles.tile([P, 64], bf16, tag=f"w{c}", name=f"w_sb{c}")
                for c in range(NCH)
            ]
            bb_sb = singles.tile([P, 1], f32)
            urep_sb = singles.tile([P, P], bf16)
            num_sb = singles.tile([P, NCH * BL], f32)
            den_sb = singles.tile([P, NE * BL], f32)

            for c in range(NCH):
                nc.scalar.dma_start(out=w_sb[c][:, :], in_=w[c, :, :])
            nc.scalar.dma_start(out=bb_sb[:, :], in_=bb[:, :])
            nc.scalar.dma_start(out=urep_sb[:, :], in_=urep[:, :])

            x_pools = (x0_pool, x1_pool)
            def batch(b):
                # DMA both chunk tiles for this batch (1MB each). The first
                # batch splits into 1024-col pieces, interleaved across the
                # chunks, so stage1 of the first token group can start after
                # ~0.5MB instead of 2MB (shorter pipeline ramp).
                xt_t = []
                for c in range(NCH):
                    t_ = x_pools[c].tile([P, T], bf16, tag=f"xt{c}", name=f"xt{c}")
                    xt_t.append(t_)
                if b == 0:
                    # 1024-col pieces so stage1 starts after ~0.5MB
                    for k in range(NE):
                        for c in range(NCH):
                            nc.sync.dma_start(
                                out=xt_t[c][:, k * 1024 : (k + 1) * 1024],
                                in_=xt[c, :, k * 1024 : (k + 1) * 1024],
                            )
                else:
                    for c in range(NCH):
                        nc.sync.dma_start(
                            out=xt_t[c][:, :], in_=xt[c, :, b * T : (b + 1) * T]
                        )
                e_sb = e_pool.tile([P, T], bf16, tag="e", name="e_sb")
                uitT = [None, None]
                for k in range(NE):  # e-tile = 1024 tokens
                    g, r = divmod(k, 2)
                    if r == 0:
                        # stage1 for token group g (2048 tokens, stacked)
                        U = ps_uit_pool.tile([P, 1024], f32, tag="U", name="U")
                        for rr in range(2):
                            t0 = g * 2048 + rr * 1024
                            for c in range(NCH):
                                # even 512-block -> partitions 0:64
                                nc.tensor.matmul(
                                    U[0:64, rr * 512 : (rr + 1) * 512],
                                    lhsT=w_sb[c][:, :],
                                    rhs=xt_t[c][:, t0 : t0 + 512],
                                    start=(c == 0),
                                    stop=(c == NCH - 1),
                                )
                                # odd 512-block -> partitions 64:128
                                nc.tensor.matmul(
                                    U[64:128, rr * 512 : (rr + 1) * 512],
                                    lhsT=w_sb[c][:, :],
                                    rhs=xt_t[c][:, t0 + 512 : t0 + 1024],
                                    start=(c == 0),
                                    stop=(c == NCH - 1),
                                )
                        uitT[g] = uitT_pool.tile([P, 1024], bf16, tag="uitT", name="uitT")
                        nc.scalar.activation(
                            uitT[g][:, :], U[:, :], Tanh, bias=bb_sb[:, :]
                        )
                    # stage2: two row-packed matmuls -> [128, 1024] logits
                    E = ps_e_pool.tile([P, 1024], f32, tag="E", name="E")
                    nc.tensor.matmul(
                        E[:, 0:512],
                        lhsT=urep_sb[0:64, :],
                        rhs=uitT[g][0:64, r * 512 : (r + 1) * 512],
                        start=True,
                        stop=True,
                    )
                    nc.tensor.matmul(
                        E[:, 512:1024],
                        lhsT=urep_sb[64:128, :],
                        rhs=uitT[g][64:128, r * 512 : (r + 1) * 512],
                        start=True,
                        stop=True,
                    )
                    nc.scalar.activation(
                        e_sb[:, k * 1024 : (k + 1) * 1024], E[:, :], Exp,
                        accum_out=den_sb[:, b * NE + k : b * NE + k + 1],
                    )
                # numerator TTRs on DVE (the bottleneck engine).
                # batch 0 runs per-e-tile with seed chaining so the DVE can
                # start before the whole batch's e is ready (shorter ramp);
                # later batches use one big TTR per chunk (lower overhead).
                if b == 0:
                    for c in range(NCH):
                        col = b * NCH + c
                        for k in range(NE):
                            scr = scratch_pool.tile(
                                [P, 1024], bf16, tag=f"scr{c}", name="scr_k"
                            )
                            nc.vector._custom_dve(
                                TENSOR_TENSOR_REDUCE,
                                out=scr[:, :],
                                in0=xt_t[c][:, k * 1024 : (k + 1) * 1024],
                                in1=e_sb[:, k * 1024 : (k + 1) * 1024],
                                s0=0.0 if k == 0 else num_sb[:, col : col + 1],
                                s1=1.0,
                                accum_out=num_sb[:, col : col + 1],
                            )
                else:
                    for c in range(NCH):
                        col = b * NCH + c
                        scr = scratch_pool.tile(
                            [P, T], bf16, tag=f"scr{c}", name="scr_b"
                        )
                        nc.vector._custom_dve(
                            TENSOR_TENSOR_REDUCE,
                            out=scr[:, :],
                            in0=xt_t[c][:, :],
                            in1=e_sb[:, :],
                            s0=0.0,
                            s1=1.0,
                            accum_out=num_sb[:, col : col + 1],
                        )

            for b in range(BL):
                batch(b)
                # drain this batch's numerator right away: keeps the tail
                # DMA tiny and avoids a bulk sync point near the end
                nc.sync.dma_start(
                    out=num[:, b * NCH : (b + 1) * NCH],
                    in_=num_sb[:, b * NCH : (b + 1) * NCH],
                )
                if b == BL - 2:
                    # den partials for batches 0..6 are final by now
                    nc.sync.dma_start(
                        out=den[:, : (BL - 1) * NE],
                        in_=den_sb[0:1, : (BL - 1) * NE],
                    )
            nc.sync.dma_start(
                out=den[:, (BL - 1) * NE :], in_=den_sb[0:1, (BL - 1) * NE :]
            )
    return nc


def _declare(nc):
    f32 = mybir.dt.float32
    bf16 = mybir.dt.bfloat16
    aps = {
        "xt": nc.dram_tensor("xt", (NCH, P, BL * T), bf16, kind="ExternalInput").ap(),
        "w": nc.dram_tensor("w", (NCH, P, 64), bf16, kind="ExternalInput").ap(),
        "bb": nc.dram_tensor("bb", (P, 1), f32, kind="ExternalInput").ap(),
        "urep": nc.dram_tensor("urep", (P, P), bf16, kind="ExternalInput").ap(),
        "num": nc.dram_tensor("num", (P, NCH * BL), f32, kind="ExternalOutput").ap(),
        "den": nc.dram_tensor("den", (1, NE * BL), f32, kind="ExternalOutput").ap(),
    }
    return aps


_CACHE = {}


def _get_nc():
    key = "nc"
    if key not in _CACHE:
        nc = bacc.Bacc(
            "TRN2", target_bir_lowering=False, debug=False,
            enable_asserts=False, num_devices=NCORES,
        )
        aps = _declare(nc)
        build_attpool(nc, aps)
        nc.compile()
        _CACHE[key] = nc
    return _CACHE[key]


def _host_prep(x, W, b, u):
    """Build per-core input maps from full inputs."""
    x = np.asarray(x, dtype=np.float32)
    W = np.asarray(W, dtype=np.float32)
    b = np.asarray(b, dtype=np.float32)
    u = np.asarray(u, dtype=np.float32)
    wc = np.zeros((NCH, P, 64), dtype=np.float32)
    wc[:, :, :A] = W.reshape(NCH, P, A)
    wc = wc.astype(ml_dtypes.bfloat16)
    bb = np.zeros((P, 1), dtype=np.float32)
    bb[0:A, 0] = b
    bb[64 : 64 + A, 0] = b
    urep = np.zeros((P, P), dtype=np.float32)
    urep[0:A, :] = u.reshape(A, 1)
    urep[64 : 64 + A, :] = u.reshape(A, 1)
    urep = urep.astype(ml_dtypes.bfloat16)
    in_maps = []
    for core in range(NCORES):
        xc = x[core * BL : (core + 1) * BL]  # [BL, T, D]
        # -> [NCH, P, BL*T]: xt[c, dp, b*T+t] = x[b, t, c*128+dp]
        xt = np.ascontiguousarray(
            xc.reshape(BL, T, NCH, P).transpose(2, 3, 0, 1).reshape(NCH, P, BL * T)
        ).astype(ml_dtypes.bfloat16)
        in_maps.append({"xt": xt, "w": wc, "bb": bb, "urep": urep})
    return in_maps


def _unshard(results):
    out = np.empty((B, D), dtype=np.float32)
    for core in range(NCORES):
        num = results[core]["num"]          # [128, 2*BL]
        den = results[core]["den"]          # [1, NE*BL]
        den_b = den.reshape(BL, NE).sum(axis=1)  # [BL]
        for bl in range(BL):
            vec = np.concatenate(
                [num[:, bl * NCH + c] for c in range(NCH)]
            )  # [D]
            out[core * BL + bl] = vec / (den_b[bl] + EPS)
    return out


def kernel(x, W, b, u, _trace=False):
    nc = _get_nc()
    in_maps = _host_prep(x, W, b, u)
    res = bass_utils.run_bass_kernel_spmd(
        nc, in_maps, core_ids=list(range(NCORES)), trace=_trace,
    )
    out = _unshard(res.results)
    if _trace:
        kernel.last_result = res
    return out


# revision 31
# speedup vs baseline: 1.0387x; 1.0237x over previous
"""Attention-pooling kernel (AttLayer) for Trainium2, 8 NeuronCores.

Math (per batch b):
    uit  = tanh(x @ W + b)          # [T, A]
    ait  = exp(uit @ u)             # [T]
    out  = (sum_t ait[t] * x[t,:]) / (sum_t ait[t] + EPS)   # [D]

Single pass over x; numerator accumulated on DVE (custom
TENSOR_TENSOR_REDUCE, 1x mode -> the kernel is DVE-bound), so the
design minimizes DVE op count (one [128, 4096] TTR per (batch, d-chunk))
and keeps every other engine far below the DVE budget:

  - stage1 (uit^T) is 2-way "stacked": even 512-token blocks land on
    psum partitions 0-63, odd blocks on 64-127 (weights zero-padded to
    64 columns so all rows are written -> no NaN garbage). This halves
    ACT tanh time (free-dim is what costs on ACT).
  - stage2 (logit broadcast) is row-packed: two concurrent K=64
    matmuls (tile_position (0,0)/(64,0) inferred from base partitions)
    produce the [128, t] broadcast logits for two 512-blocks at once.
  - exp materializes e (bf16) into a [128, 4096] per-batch tile with
    accum_out -> per-e-tile denominator partials.

Device data layout (per core, pure data-parallel over batch; host casts
to bf16 and pre-transposes so x streams with d on partitions):
    xt   [2, 128, BL*T] bf16  -- x^T: (d_chunk, d_in_chunk, b*t)
    w    [2, 128, 64]   bf16  -- W d-chunks, zero-padded cols 50..63
    bb   [128, 1]       f32   -- bias stacked (rows 0-49 & 64-113 = b)
    urep [128, 128]     bf16  -- u bcast: rows 0-49 & 64-113 = u, else 0
Outputs:
    num  [128, 2*BL]    f32   -- numerator, col = b*2 + c
    den  [1, 4*BL]      f32   -- per-e-tile exp-sum partials, col = b*4+k
Final division num/(den+EPS) happens on host during unsharding.
"""

import sys
import types

sys.path.insert(0, "/opt/trn_rl_repo")

# bass_utils' trace path imports antenv.axon_hooks, which not every image
# ships; register a no-op fallback so trace=True degrades instead of crashing.
try:
    import antenv.axon_hooks  # noqa: F401
except ImportError:
    try:
        import antenv

        _hooks = types.ModuleType("antenv.axon_hooks")
        _hooks._HOOK = None

        def _set_hook(hook):
            _hooks._HOOK = hook

        def _get_hook():
            return _hooks._HOOK

        _hooks.set_axon_ntff_profile_hook = _set_hook
        _hooks.get_axon_ntff_profile_hook = _get_hook
        sys.modules["antenv.axon_hooks"] = _hooks
        antenv.axon_hooks = _hooks
    except ImportError:
        pass

import numpy as np
import ml_dtypes

import concourse.bacc as bacc
import concourse.tile as tile
from concourse import mybir
from concourse import bass_utils
from concourse.dve_ops import TENSOR_TENSOR_REDUCE

B, T, D, A = 64, 4096, 256, 50
NCORES = 8
BL = B // NCORES  # batches per core
EPS = 1e-7
P = 128
NCH = D // P  # 2 d-chunks
NE = 4        # e-tiles (of 1024 tokens) per batch
# Batches whose numerator runs on the Tensor engine (from a second,
# natural-layout x stream) instead of the DVE: the DVE's 1x-mode
# TENSOR_TENSOR_REDUCE is the kernel bottleneck, so offloading 2 of 8
# batches re-balances DVE (~54us) against DMA (~53us) / ACT (~56us).
# Disabled (PEB = ()): measured 101.5us vs 98.6us for the pure-DVE
# pipeline, and first-run-after-compile reliability was worse. The
# machinery is kept behind the PEB flag for future tuning.
PEB = ()
NPEB = len(PEB)
NJ = T // P   # 32 token groups of 128 per batch


def build_attpool(nc, aps):
    xt, w, bb, urep = aps["xt"], aps["w"], aps["bb"], aps["urep"]
    num, den = aps["num"], aps["den"]
    f32 = mybir.dt.float32
    bf16 = mybir.dt.bfloat16
    Tanh = mybir.ActivationFunctionType.Tanh
    Exp = mybir.ActivationFunctionType.Exp

    with tile.TileContext(nc) as tc:
        with (
            tc.tile_pool(name="singles", bufs=1) as singles,
            tc.tile_pool(name="x0", bufs=4) as x0_pool,
            tc.tile_pool(name="x1", bufs=4) as x1_pool,
            tc.tile_pool(name="uitT", bufs=2) as uitT_pool,
            tc.tile_pool(name="e", bufs=3) as e_pool,
            tc.tile_pool(name="scratch", bufs=3) as scratch_pool,
            tc.tile_pool(name="ps_uit", bufs=2, space="PSUM") as ps_uit_pool,
            tc.tile_pool(name="ps_e", bufs=2, space="PSUM") as ps_e_pool,
        ):
            # constants (tiles allocated first so SBUF layout is stable;
            # their DMA issues go out after the first x pieces -- see below)
            w_sb = [
                singles.tile([P, 64], bf16, tag=f"w{c}", name=f"w_sb{c}")
                for c in range(NCH)
            ]
            bb_sb = singles.tile([P, 1], f32)
            urep_sb = singles.tile([P, P], bf16)
            num_sb = singles.tile([P, NCH * BL], f32)
            den_sb = singles.tile([P, NE * BL], f32)

            x_pools = (x0_pool, x1_pool)
            # batch 0's first 1024-col pieces issue on the Scalar queue
            # AHEAD of the consts: the x stream is the critical path
            xt0_t = []
            for c in range(NCH):
                t_ = x_pools[c].tile([P, T], bf16, tag=f"xt{c}", name=f"xt{c}")
                xt0_t.append(t_)
                nc.scalar.dma_start(out=t_[:, 0:1024], in_=xt[c, :, 0:1024])
            for c in range(NCH):
                nc.scalar.dma_start(out=w_sb[c][:, :], in_=w[c, :, :])
            nc.scalar.dma_start(out=bb_sb[:, :], in_=bb[:, :])
            nc.scalar.dma_start(out=urep_sb[:, :], in_=urep[:, :])
            def batch(b):
                # DMA both chunk tiles for this batch (1MB each). The first
                # batch splits into 1024-col pieces, interleaved across the
                # chunks, so stage1 of the first token group can start after
                # ~0.5MB instead of 2MB (shorter pipeline ramp).
                if b == 0:
                    # tiles pre-created (first pieces already in flight on
                    # the Scalar queue); stream the rest in 1024-col pieces
                    xt_t = xt0_t
                    for k in range(1, NE):
                        for c in range(NCH):
                            nc.sync.dma_start(
                                out=xt_t[c][:, k * 1024 : (k + 1) * 1024],
                                in_=xt[c, :, k * 1024 : (k + 1) * 1024],
                            )
                else:
                    xt_t = []
                    for c in range(NCH):
                        t_ = x_pools[c].tile(
                            [P, T], bf16, tag=f"xt{c}", name=f"xt{c}"
                        )
                        xt_t.append(t_)
                    for c in range(NCH):
                        nc.sync.dma_start(
                            out=xt_t[c][:, :], in_=xt[c, :, b * T : (b + 1) * T]
                        )
                e_sb = e_pool.tile([P, T], bf16, tag="e", name="e_sb")
                uitT = [None, None]
                for k in range(NE):  # e-tile = 1024 tokens
                    g, r = divmod(k, 2)
                    if r == 0:
                        # stage1 for token group g (2048 tokens, stacked)
                        U = ps_uit_pool.tile([P, 1024], f32, tag="U", name="U")
                        for rr in range(2):
                            t0 = g * 2048 + rr * 1024
                            for c in range(NCH):
                                # even 512-block -> partitions 0:64
                                nc.tensor.matmul(
                                    U[0:64, rr * 512 : (rr + 1) * 512],
                                    lhsT=w_sb[c][:, :],
                                    rhs=xt_t[c][:, t0 : t0 + 512],
                                    start=(c == 0),
                                    stop=(c == NCH - 1),
                                )
                                # odd 512-block -> partitions 64:128
                                nc.tensor.matmul(
                                    U[64:128, rr * 512 : (rr + 1) * 512],
                                    lhsT=w_sb[c][:, :],
                                    rhs=xt_t[c][:, t0 + 512 : t0 + 1024],
                                    start=(c == 0),
                                    stop=(c == NCH - 1),
                                )
                        uitT[g] = uitT_pool.tile([P, 1024], bf16, tag="uitT", name="uitT")
                        nc.scalar.activation(
                            uitT[g][:, :], U[:, :], Tanh, bias=bb_sb[:, :]
                        )
                    # stage2: two row-packed matmuls -> [128, 1024] logits
                    E = ps_e_pool.tile([P, 1024], f32, tag="E", name="E")
                    nc.tensor.matmul(
                        E[:, 0:512],
                        lhsT=urep_sb[0:64, :],
                        rhs=uitT[g][0:64, r * 512 : (r + 1) * 512],
                        start=True,
                        stop=True,
                    )
                    nc.tensor.matmul(
                        E[:, 512:1024],
                        lhsT=urep_sb[64:128, :],
                        rhs=uitT[g][64:128, r * 512 : (r + 1) * 512],
                        start=True,
                        stop=True,
                    )
                    nc.scalar.activation(
                        e_sb[:, k * 1024 : (k + 1) * 1024], E[:, :], Exp,
                        accum_out=den_sb[:, b * NE + k : b * NE + k + 1],
                    )
                # numerator TTRs on DVE (the bottleneck engine).
                # batch 0 runs per-e-tile with seed chaining so the DVE can
                # start before the whole batch's e is ready (shorter ramp);
                # later batches use one big TTR per chunk (lower overhead).
                if b <= 1:
                    for c in range(NCH):
                        col = b * NCH + c
                        for k in range(NE):
                            scr = scratch_pool.tile(
                                [P, 1024], bf16, tag=f"scr{c}", name="scr_k"
                            )
                            nc.vector._custom_dve(
                                TENSOR_TENSOR_REDUCE,
                                out=scr[:, :],
                                in0=xt_t[c][:, k * 1024 : (k + 1) * 1024],
                                in1=e_sb[:, k * 1024 : (k + 1) * 1024],
                                s0=0.0 if k == 0 else num_sb[:, col : col + 1],
                                s1=1.0,
                                accum_out=num_sb[:, col : col + 1],
                            )
                else:
                    for c in range(NCH):
                        col = b * NCH + c
                        scr = scratch_pool.tile(
                            [P, T], bf16, tag=f"scr{c}", name="scr_b"
                        )
                        nc.vector._custom_dve(
                            TENSOR_TENSOR_REDUCE,
                            out=scr[:, :],
                            in0=xt_t[c][:, :],
                            in1=e_sb[:, :],
                            s0=0.0,
                            s1=1.0,
                            accum_out=num_sb[:, col : col + 1],
                        )

            for b in range(BL):
                batch(b)
                # drain this batch's numerator right away: keeps the tail
                # DMA tiny and avoids a bulk sync point near the end
                nc.sync.dma_start(
                    out=num[:, b * NCH : (b + 1) * NCH],
                    in_=num_sb[:, b * NCH : (b + 1) * NCH],
                )
                if b == BL - 2:
                    # den partials for batches 0..6 are final by now
                    nc.sync.dma_start(
                        out=den[:, : (BL - 1) * NE],
                        in_=den_sb[0:1, : (BL - 1) * NE],
                    )
            nc.sync.dma_start(
                out=den[:, (BL - 1) * NE :], in_=den_sb[0:1, (BL - 1) * NE :]
            )
    return nc


def _declare(nc):
    f32 = mybir.dt.float32
    bf16 = mybir.dt.bfloat16
    aps = {
        "xt": nc.dram_tensor("xt", (NCH, P, BL * T), bf16, kind="ExternalInput").ap(),
        "w": nc.dram_tensor("w", (NCH, P, 64), bf16, kind="ExternalInput").ap(),
        "bb": nc.dram_tensor("bb", (P, 1), f32, kind="ExternalInput").ap(),
        "urep": nc.dram_tensor("urep", (P, P), bf16, kind="ExternalInput").ap(),
        "num": nc.dram_tensor("num", (P, NCH * BL), f32, kind="ExternalOutput").ap(),
        "den": nc.dram_tensor("den", (1, NE * BL), f32, kind="ExternalOutput").ap(),
    }
    return aps


_CACHE = {}


def _get_nc():
    key = "nc"
    if key not in _CACHE:
        nc = bacc.Bacc(
            "TRN2", target_bir_lowering=False, debug=False,
            enable_asserts=False, num_devices=NCORES,
        )
        aps = _declare(nc)
        build_attpool(nc, aps)
        nc.compile()
        _CACHE[key] = nc
    return _CACHE[key]


def _host_prep(x, W, b, u):
    """Build per-core input maps from full inputs."""
    x = np.asarray(x, dtype=np.float32)
    W = np.asarray(W, dtype=np.float32)
    b = np.asarray(b, dtype=np.float32)
    u = np.asarray(u, dtype=np.float32)
    wc = np.zeros((NCH, P, 64), dtype=np.float32)
    wc[:, :, :A] = W.reshape(NCH, P, A)
    wc = wc.astype(ml_dtypes.bfloat16)
    bb = np.zeros((P, 1), dtype=np.float32)
    bb[0:A, 0] = b
    bb[64 : 64 + A, 0] = b
    urep = np.zeros((P, P), dtype=np.float32)
    urep[0:A, :] = u.reshape(A, 1)
    urep[64 : 64 + A, :] = u.reshape(A, 1)
    urep = urep.astype(ml_dtypes.bfloat16)
    in_maps = []
    for core in range(NCORES):
        xc = x[core * BL : (core + 1) * BL]  # [BL, T, D]
        # -> [NCH, P, BL*T]: xt[c, dp, b*T+t] = x[b, t, c*128+dp]
        xt = np.ascontiguousarray(
            xc.reshape(BL, T, NCH, P).transpose(2, 3, 0, 1).reshape(NCH, P, BL * T)
        ).astype(ml_dtypes.bfloat16)
        in_maps.append({"xt": xt, "w": wc, "bb": bb, "urep": urep})
    return in_maps


def _unshard(results):
    out = np.empty((B, D), dtype=np.float32)
    for core in range(NCORES):
        num = results[core]["num"]          # [128, 2*BL]
        den = results[core]["den"]          # [1, NE*BL]
        den_b = den.reshape(BL, NE).sum(axis=1)  # [BL]
        for bl in range(BL):
            vec = np.concatenate(
                [num[:, bl * NCH + c] for c in range(NCH)]
            )  # [D]
            out[core * BL + bl] = vec / (den_b[bl] + EPS)
    return out


def kernel(x, W, b, u, _trace=False):
    nc = _get_nc()
    in_maps = _host_prep(x, W, b, u)
    res = bass_utils.run_bass_kernel_spmd(
        nc, in_maps, core_ids=list(range(NCORES)), trace=_trace,
    )
    out = _unshard(res.results)
    if _trace:
        kernel.last_result = res
    return out
